# revision 11
# baseline (speedup 1.0000x reference)
"""Distributed Trainium2 Bass kernel for fused LayerNorm + causal multi-head
attention + output projection (B=2, T=2048, DIM=1024, H=16, D=64) on 8 cores.

Sharding:
  - LayerNorm + final projection: token-parallel (512 tokens/core).
  - QKV projection + attention: head-parallel (2 heads x 2 batches per core).
  - xn travels via bf16 AllGather; attention outputs via bf16 AllToAll.

Compute dtype: bf16 matmuls with fp32 PSUM accumulation (rel err ~5e-3).
All LN affine params and the 1/sqrt(D) score scale are folded into the QKV
weights on the host.
"""
import os
import sys
import types
import numpy as np
import ml_dtypes

# ---------------------------------------------------------------- constants
B, T, DIM, D = 2, 2048, 1024, 64
H = DIM // D            # 16 heads
NC = 8                  # cores
TOK = B * T             # 4096 tokens
TPC = TOK // NC         # 512 tokens per core
KT8 = DIM // 128        # 8 contraction tiles
EPS = 1e-5

TRACE = bool(int(os.environ.get("BASS_KERNEL_TRACE", "0")))

BF16_NP = ml_dtypes.bfloat16


def _ensure_ntff_hook():
    """The agent image lacks antenv.axon_hooks; recreate it so trace=True works."""
    if "antenv.axon_hooks" not in sys.modules:
        mod = types.ModuleType("antenv.axon_hooks")
        mod._hook = None
        def set_axon_ntff_profile_hook(h):
            mod._hook = h
        def get_axon_ntff_profile_hook():
            return mod._hook
        mod.set_axon_ntff_profile_hook = set_axon_ntff_profile_hook
        mod.get_axon_ntff_profile_hook = get_axon_ntff_profile_hook
        sys.modules["antenv.axon_hooks"] = mod
    m = sys.modules["antenv.axon_hooks"]
    if m.get_axon_ntff_profile_hook() is None:
        try:
            from trn_agent_boot.trn_boot import _ntff_profile_via_ctypes
            m.set_axon_ntff_profile_hook(
                _ntff_profile_via_ctypes("/opt/axon/libaxon_pjrt.so"))
        except Exception:
            pass


def build_graph():
    import concourse.bass as bass
    import concourse.bacc as bacc
    import concourse.tile as tile
    import concourse.mybir as mybir

    dt = mybir.dt
    F32, BF16 = dt.float32, dt.bfloat16
    AF = mybir.ActivationFunctionType
    ALU = mybir.AluOpType
    RG = [list(range(NC))]

    nc = bacc.Bacc(None, target_bir_lowering=False, debug=False, num_devices=NC)

    # ------------------------------------------------------------ I/O
    x_in = nc.dram_tensor("x_c", [TPC, DIM], F32, kind="ExternalInput")
    wt_in = nc.dram_tensor("wt_c", [DIM, 384], BF16, kind="ExternalInput")
    bias_in = nc.dram_tensor("bias_c", [128, 3], F32, kind="ExternalInput")
    pwt_in = nc.dram_tensor("pwt", [DIM, DIM], BF16, kind="ExternalInput")
    pb_in = nc.dram_tensor("pb", [1, DIM], BF16, kind="ExternalInput")
    idn_in = nc.dram_tensor("idn", [128, 128], BF16, kind="ExternalInput")
    ones_in = nc.dram_tensor("ones_r", [1, 128], BF16, kind="ExternalInput")
    emat_in = nc.dram_tensor("emat", [33, 128], BF16, kind="ExternalInput")
    out_dram = nc.dram_tensor("out_c", [TPC, DIM], F32, kind="ExternalOutput")

    with tile.TileContext(nc) as tc:
        with (
            tc.tile_pool(name="persist", bufs=1) as pers,
            tc.tile_pool(name="work", bufs=2) as work,
            tc.tile_pool(name="dram", bufs=1, space="DRAM") as dram,
        ):
            # ---------------- DRAM bounce buffers ----------------
            ag_in = dram.tile([DIM, TPC], BF16)
            ag_out = dram.tile([NC * DIM, TPC], BF16, addr_space="Shared")
            a2a_in = dram.tile([NC * 128, TPC], BF16)
            a2a_out = dram.tile([NC * 128, TPC], BF16)

            # ================= P1: LayerNorm (token slice, natural) ========
            xn_sb = pers.tile([128, 4 * DIM], BF16)   # 4 token tiles side by side
            with tc.tile_pool(name="ln", bufs=2) as lnp:
                for t in range(4):
                    xt = lnp.tile([128, DIM], F32, tag="xt")
                    nc.sync.dma_start(xt[:], x_in[128 * t:128 * (t + 1), :])
                    nmu = lnp.tile([128, 1], F32, tag="nmu")
                    musum = lnp.tile([128, 1], F32, tag="musum")
                    nc.vector.reduce_sum(musum[:], xt[:], axis=mybir.AxisListType.X)
                    nc.vector.tensor_scalar_mul(nmu[:], musum[:], -1.0 / DIM)
                    sq_dump = lnp.tile([128, DIM], BF16, tag="sqd")
                    sumsq = lnp.tile([128, 1], F32, tag="sumsq")
                    nc.scalar.activation(sq_dump[:], xt[:], AF.Square,
                                         bias=nmu[:], scale=1.0,
                                         accum_out=sumsq[:])
                    vareps = lnp.tile([128, 1], F32, tag="vareps")
                    nc.vector.tensor_scalar(vareps[:], sumsq[:], 1.0 / DIM, EPS,
                                            op0=ALU.mult, op1=ALU.add)
                    std = lnp.tile([128, 1], F32, tag="std")
                    nc.scalar.activation(std[:], vareps[:], AF.Sqrt)
                    rstd = lnp.tile([128, 1], F32, tag="rstd")
                    nc.vector.reciprocal(rstd[:], std[:])
                    nmr = lnp.tile([128, 1], F32, tag="nmr")
                    nc.vector.scalar_tensor_tensor(
                        nmr[:], nmu[:], 1.0, rstd[:],
                        op0=ALU.mult, op1=ALU.mult)
                    nc.scalar.activation(xn_sb[:, DIM * t:DIM * (t + 1)], xt[:],
                                         AF.Identity, bias=nmr[:], scale=rstd[:])

            # ---------------- constant / weight loads (after x: LN first) ----
            idn_sb = pers.tile([128, 128], BF16)
            nc.sync.dma_start(idn_sb[:], idn_in[:])
            wt_sb = pers.tile([128, KT8 * 384], BF16)       # k-major qkv weights
            nc.sync.dma_start(
                wt_sb[:].rearrange("p (k o) -> p k o", o=384),
                wt_in[:].rearrange("(k p) o -> p k o", p=128),
            )
            bias_sb = pers.tile([128, 3], F32)
            nc.sync.dma_start(bias_sb[:], bias_in[:])
            pwt_sb = pers.tile([128, KT8 * DIM], BF16)      # k-major proj weights
            nc.sync.dma_start(
                pwt_sb[:].rearrange("p (k o) -> p k o", o=DIM),
                pwt_in[:].rearrange("(k p) o -> p k o", p=128),
            )
            pb_sb = pers.tile([1, DIM], BF16)
            nc.sync.dma_start(pb_sb[:], pb_in[:])
            ones_sb = pers.tile([1, 128], BF16)
            nc.sync.dma_start(ones_sb[:], ones_in[:])
            emat_sb = pers.tile([33, 128], BF16)
            nc.sync.dma_start(emat_sb[:], emat_in[:])
            sums_col = pers.tile([33, 512], F32)
            nc.vector.memset(sums_col[:], 1.0)

            # ================= P2: transpose xn -> xnT, stage AG input =====
            xnT_sb = pers.tile([128, KT8 * TPC], BF16)  # [dim-tile partition, k*512+t128]
            with tc.tile_pool(name="ps_tr", bufs=6, space="PSUM") as pstr:
                for t in range(4):
                    for k in range(KT8):
                        trp = pstr.tile([128, 128], BF16, tag="tr")
                        nc.tensor.transpose(
                            trp[:], xn_sb[:, DIM * t + 128 * k: DIM * t + 128 * (k + 1)],
                            idn_sb[:])
                        nc.vector.tensor_copy(
                            xnT_sb[:, TPC * k + 128 * t: TPC * k + 128 * (t + 1)],
                            trp[:])
                for k in range(KT8):
                    nc.sync.dma_start(ag_in[128 * k:128 * (k + 1), :],
                                      xnT_sb[:, TPC * k:TPC * (k + 1)])

            # ================= P3: AllGather xnT ===========================
            nc.gpsimd.collective_compute(
                "AllGather", ALU.bypass, replica_groups=RG,
                ins=[ag_in[:].opt()], outs=[ag_out[:].opt()],
            )

            # ================= P4: QKV projection (head slice, all tokens) =
            qkvT = []
            for name in ("qT", "kT", "vT"):
                t_ = pers.tile([128, TOK], BF16, name=name)
                qkvT.append(t_)
            with (
                tc.tile_pool(name="qkv_x", bufs=3) as qxp,
                tc.tile_pool(name="ps_qkv", bufs=3, space="PSUM") as psq,
            ):
                for r in range(NC):
                    xr = []
                    for k in range(KT8):
                        xk = qxp.tile([128, TPC], BF16, tag=f"xr{k % 2}")
                        nc.sync.dma_start(
                            xk[:], ag_out[DIM * r + 128 * k: DIM * r + 128 * (k + 1), :])
                        xr.append(xk)
                    for g in range(3):
                        psg = psq.tile([128, TPC], F32, tag="qkv")
                        for k in range(KT8):
                            nc.tensor.matmul(
                                psg[:],
                                wt_sb[:, 384 * k + 128 * g: 384 * k + 128 * (g + 1)],
                                xr[k][:],
                                start=(k == 0), stop=(k == KT8 - 1))
                        nc.vector.tensor_scalar(
                            qkvT[g][:, TPC * r:TPC * (r + 1)], psg[:],
                            bias_sb[:, g:g + 1], None, op0=ALU.add)
            qT_sb, kT_sb, vT_sb = qkvT

            # ================= P5: V -> natural layout w/ ones columns =====
            vnat = []
            for b in range(B):
                vb = pers.tile([128, 16 * 130], BF16, name=f"vnat{b}")
                nc.vector.memset(
                    vb[:].rearrange("p (j a w) -> p j a w", a=2, w=65)[:, :, :, 64:65], 1.0)
                vnat.append(vb)
            with tc.tile_pool(name="ps_vtr", bufs=4, space="PSUM") as psv:
                for b in range(B):
                    for j in range(16):
                        vtr = psv.tile([128, 128], BF16, tag="vtr")
                        nc.tensor.transpose(
                            vtr[:],
                            vT_sb[:, b * T + 128 * j: b * T + 128 * (j + 1)],
                            idn_sb[:])
                        nc.vector.tensor_copy(
                            vnat[b][:, 130 * j: 130 * j + 64], vtr[:, 0:64])
                        nc.vector.tensor_copy(
                            vnat[b][:, 130 * j + 65: 130 * j + 129], vtr[:, 64:128])

            # ================= P6: causal attention (2 heads, 2 batches) ===
            # Both heads share one 2-bank PSUM tile per stage: scores s2 =
            # [128, 1024] (A cols 0:512, B cols 512:1024); one exp + one
            # affine_select covers both heads; PV accumulates into pv2
            # [65, 1024] (A | B).
            attnT = pers.tile([128, TOK], BF16)
            with (
                tc.tile_pool(name="pt", bufs=3) as ptp,
                tc.tile_pool(name="ps_s", bufs=2, space="PSUM") as pss,
                tc.tile_pool(name="ps_pv", bufs=2, space="PSUM") as psp,
                tc.tile_pool(name="sm", bufs=2) as smp,
            ):
                for b in range(B):
                    for qc in range(4):
                        q0 = b * T + 512 * qc
                        pv2 = psp.tile([65, 1024], F32, tag="pv")
                        nkp = 4 * qc + 4
                        pend = None  # software pipeline: PV lags QK by one kp
                        for kp in range(nkp):
                            k0 = b * T + 128 * kp
                            s2 = pss.tile([128, 1024], F32, tag="s")
                            nc.tensor.matmul(s2[:, 0:512],
                                             kT_sb[0:64, k0:k0 + 128],
                                             qT_sb[0:64, q0:q0 + 512],
                                             start=True, stop=True)
                            nc.tensor.matmul(s2[:, 512:1024],
                                             kT_sb[64:128, k0:k0 + 128],
                                             qT_sb[64:128, q0:q0 + 512],
                                             start=True, stop=True)
                            if pend is not None:
                                pkp, pp = pend
                                nc.tensor.matmul(pv2[:, 0:512],
                                                 vnat[b][:, 130 * pkp:130 * pkp + 65],
                                                 pp[:, 0:512],
                                                 start=(pkp == 0), stop=False)
                                nc.tensor.matmul(pv2[:, 512:1024],
                                                 vnat[b][:, 130 * pkp + 65:130 * pkp + 130],
                                                 pp[:, 512:1024],
                                                 start=(pkp == 0), stop=False)
                            p2 = ptp.tile([128, 1024], BF16, tag="p2")
                            nc.scalar.activation(p2[:], s2[:], AF.Exp)
                            if kp >= 4 * qc:  # diagonal-crossing tile
                                base = 512 * qc - 128 * kp
                                nc.gpsimd.affine_select(
                                    p2[:], p2[:], pattern=[[0, 2], [1, 512]],
                                    compare_op=ALU.is_ge, fill=0.0,
                                    base=base, channel_multiplier=-1)
                            pend = (kp, p2)
                        pkp, pp = pend
                        nc.tensor.matmul(pv2[:, 0:512],
                                         vnat[b][:, 130 * pkp:130 * pkp + 65],
                                         pp[:, 0:512],
                                         start=(pkp == 0), stop=True)
                        nc.tensor.matmul(pv2[:, 512:1024],
                                         vnat[b][:, 130 * pkp + 65:130 * pkp + 130],
                                         pp[:, 512:1024],
                                         start=(pkp == 0), stop=True)
                        # normalize: fast recip of sums -> E-matrix bcast
                        nc.vector.tensor_copy(sums_col[0:1, :],
                                              pv2[64:65, 0:512])
                        nc.vector.tensor_copy(sums_col[32:33, :],
                                              pv2[64:65, 512:1024])
                        rec = smp.tile([33, 512], F32, tag="rec")
                        nc.vector.reciprocal_approx_fast(rec[:], sums_col[:])
                        recb = smp.tile([33, 512], BF16, tag="recb")
                        nc.vector.tensor_copy(recb[:], rec[:])
                        bc2 = pss.tile([128, 1024], F32, tag="s")
                        nc.tensor.matmul(bc2[:, 0:512], emat_sb[:], recb[:],
                                         start=True, stop=True)
                        bc2s = smp.tile([128, 512], BF16, tag="bc2s")
                        nc.scalar.activation(bc2s[:], bc2[:, 0:512], AF.Identity,
                                             bias=0.0)
                        nc.vector.tensor_tensor(
                            attnT[0:64, q0:q0 + 512], pv2[0:64, 0:512],
                            bc2s[0:64, :], op=ALU.mult)
                        nc.vector.tensor_tensor(
                            attnT[64:128, q0:q0 + 512], pv2[0:64, 512:1024],
                            bc2s[64:128, :], op=ALU.mult)

            # ================= P7: AllToAll attention outputs ==============
            for r in range(NC):
                nc.sync.dma_start(a2a_in[128 * r:128 * (r + 1), :],
                                  attnT[:, TPC * r:TPC * (r + 1)])
            nc.gpsimd.collective_compute(
                "AllToAll", ALU.bypass, replica_groups=RG,
                ins=[a2a_in[:].opt()], outs=[a2a_out[:].opt()],
            )

            # ================= P8: output projection (token slice) =========
            with (
                tc.tile_pool(name="projx", bufs=1) as pxp,
                tc.tile_pool(name="ps_o", bufs=3, space="PSUM") as pso,
                tc.tile_pool(name="outp", bufs=2) as outp,
            ):
                aT = []
                for ck in range(KT8):
                    ak = pxp.tile([128, TPC], BF16, tag=f"aT{ck}")
                    nc.sync.dma_start(ak[:],
                                      a2a_out[128 * ck:128 * (ck + 1), :])
                    aT.append(ak)
                for tt in range(4):
                    ot = outp.tile([128, DIM], F32, tag="ot")
                    for half in range(2):
                        pso_t = pso.tile([128, 512], F32, tag="po")
                        for ck in range(KT8):
                            nc.tensor.matmul(
                                pso_t[:],
                                aT[ck][:, 128 * tt:128 * (tt + 1)],
                                pwt_sb[:, DIM * ck + 512 * half:
                                       DIM * ck + 512 * (half + 1)],
                                start=(ck == 0), stop=False)
                        nc.tensor.matmul(
                            pso_t[:], ones_sb[0:1, :],
                            pb_sb[:, 512 * half:512 * (half + 1)],
                            start=False, stop=True)
                        nc.vector.tensor_copy(
                            ot[:, 512 * half:512 * (half + 1)], pso_t[:])
                    nc.sync.dma_start(out_dram[128 * tt:128 * (tt + 1), :], ot[:])

    nc.compile()
    return nc


def host_prep(inputs):
    x = np.asarray(inputs["x"], np.float32).reshape(TOK, DIM)
    ln_w = np.asarray(inputs["ln_w"], np.float32)
    ln_b = np.asarray(inputs["ln_b"], np.float32)
    qkv_w = np.asarray(inputs["qkv_w"], np.float32)
    qkv_b = np.asarray(inputs["qkv_b"], np.float32)
    proj_w = np.asarray(inputs["proj_w"], np.float32)
    proj_b = np.asarray(inputs["proj_b"], np.float32)

    # fold LN affine into qkv weights; fold 1/sqrt(D) into Q rows
    Wp = qkv_w * ln_w[None, :]
    bp = qkv_b + qkv_w @ ln_b
    Wp[0:DIM] *= D ** -0.5
    bp[0:DIM] *= D ** -0.5

    idn = np.eye(128, dtype=np.float32).astype(BF16_NP)
    ones_r = np.ones((1, 128), BF16_NP)
    emat = np.zeros((33, 128), np.float32)
    emat[0, 0:64] = 1.0
    emat[32, 64:128] = 1.0
    emat = emat.astype(BF16_NP)
    pwt = proj_w.T.copy().astype(BF16_NP)
    pb = proj_b.reshape(1, DIM).astype(BF16_NP)

    in_maps = []
    for c in range(NC):
        rows = []
        for blk in range(3):
            for h in (2 * c, 2 * c + 1):
                rows.extend(range(blk * DIM + h * D, blk * DIM + (h + 1) * D))
        rows = np.array(rows)
        in_maps.append(dict(
            x_c=np.ascontiguousarray(x[TPC * c:TPC * (c + 1)]),
            wt_c=np.ascontiguousarray(Wp[rows].T).astype(BF16_NP),
            bias_c=np.ascontiguousarray(bp[rows].reshape(3, 128).T),
            pwt=pwt, pb=pb, idn=idn, ones_r=ones_r, emat=emat,
        ))
    return in_maps


_CACHED = {}


def kernel(**inputs) -> np.ndarray:
    _ensure_ntff_hook()
    from concourse import bass_utils
    if TRACE:
        bass_utils.upload_artifacts = lambda tmpdir: "/tmp/noupload"

    if "nc" not in _CACHED:
        _CACHED["nc"] = build_graph()
    nc = _CACHED["nc"]

    in_maps = host_prep(inputs)
    res = bass_utils.run_bass_kernel_spmd(
        nc, in_maps, core_ids=list(range(NC)), trace=TRACE,
        trace_cores=list(range(NC)) if TRACE else None)
    _CACHED["last_result"] = res
    out = np.concatenate([res.results[c]["out_c"] for c in range(NC)], axis=0)
    return out.reshape(B, T, DIM).astype(np.float32)
